# revision 1
# baseline (speedup 1.0000x reference)
"""Trainium2 Bass kernel for nn_AttentionFilter (B=2,C=128,H=256,W=510).

Sharding: 8 cores = 2 batches x 4 channel-groups of 32. Per core:
  1x1 conv (channel mix, M=32 matmul) -> DRAM spill -> per-channel:
  PE-transpose -> rfft_w (DFT matmul) -> fft_h (complex DFT matmul) ->
  filter mult (DVE) -> variance (bn_stats) -> freq attention matmul ->
  sigmoid (ACT, fused 1/sqrt(2pi var) row scale) -> ifft_i (transposed
  output) -> irfft_j -> residual add -> channel-LN partial stats.
  Cross-core AllReduce of LN stats within each batch group, then
  normalize. All matmuls run as float32r (full-rate fp32).
"""
import sys

sys.path.insert(0, "/opt/trn_rl_repo")

import numpy as np

import concourse.bass as bass
import concourse.mybir as mybir
import concourse.tile as tile
from concourse import bacc
from concourse.bass_utils import run_bass_kernel_spmd
from concourse.masks import make_identity

B, C, H, W = 2, 128, 256, 510
WF = 256
NCH = 32  # channels per core
N_CORES = 8
F32 = mybir.dt.float32
F32R = mybir.dt.float32r
AF = mybir.ActivationFunctionType


def _r(ap):
    return ap.bitcast(F32R)


def build_consts():
    Fw = np.fft.rfft(np.eye(W), axis=0, norm="ortho").T  # [W, WF]
    fw_pack = np.zeros((512, 512), np.float32)
    fw_pack[:W, :WF] = Fw.real
    fw_pack[:W, WF:] = Fw.imag
    DH = np.fft.fft(np.eye(H), axis=0, norm="ortho")  # [kh, h]
    dht_r = np.ascontiguousarray(DH.real.T, np.float32)  # [h, kh]
    dht_i = np.ascontiguousarray(DH.imag.T, np.float32)
    dht_ni = np.ascontiguousarray((-DH.imag).T, np.float32)
    IDH = np.fft.ifft(np.eye(H), axis=0, norm="ortho")  # [m, i]
    idht_r = np.ascontiguousarray(IDH.real.T, np.float32)  # [i, m]
    idht_i = np.ascontiguousarray(IDH.imag.T, np.float32)
    idht_ni = np.ascontiguousarray((-IDH.imag).T, np.float32)
    AR = np.fft.irfft(np.eye(WF), n=W, axis=0, norm="ortho")  # [n, j]
    AI = np.fft.irfft(1j * np.eye(WF), n=W, axis=0, norm="ortho")
    awr = np.ascontiguousarray(AR.T, np.float32)  # [j, n]
    awi = np.ascontiguousarray(AI.T, np.float32)
    return dict(fw=fw_pack, dht_r=dht_r, dht_i=dht_i, dht_ni=dht_ni,
                idht_r=idht_r, idht_i=idht_i, idht_ni=idht_ni,
                awr=awr, awi=awi)


def build_program():
    nc = bacc.Bacc("TRN2", target_bir_lowering=False, debug=False,
                   num_devices=N_CORES)

    def inp(name, shape, dt=F32):
        return nc.dram_tensor(name, list(shape), dt, kind="ExternalInput").ap()

    gb = inp("gb", (C, H, W), mybir.dt.float16)
    xb = inp("xb", (C, H, W), mybir.dt.float16)
    xres = inp("xres", (NCH, H, W))
    wgT = inp("wgT", (C, NCH), mybir.dt.float16)
    wxT = inp("wxT", (C, NCH), mybir.dt.float16)
    bg = inp("bg", (128, 1))
    bx = inp("bx", (128, 1))
    fw = inp("fw", (512, 512), F32R)
    dht_r = inp("dht_r", (H, H), F32R)
    dht_i = inp("dht_i", (H, H), F32R)
    dht_ni = inp("dht_ni", (H, H), F32R)
    idht_r = inp("idht_r", (H, H), F32R)
    idht_i = inp("idht_i", (H, H), F32R)
    idht_ni = inp("idht_ni", (H, H), F32R)
    awr = inp("awr", (WF, W), F32R)
    awi = inp("awi", (WF, W), F32R)
    fpg = inp("fpg", (NCH, 2, H, WF), mybir.dt.float16)
    fpx = inp("fpx", (NCH, 2, H, WF), mybir.dt.float16)
    gamma = inp("gamma", (1, NCH))
    beta = inp("beta", (1, NCH))
    out = nc.dram_tensor("out", [NCH, H, W], F32, kind="ExternalOutput").ap()
    dbg_zg = nc.dram_tensor("dbg_zg", [128, 2, 512], F32R,
                            kind="ExternalOutput").ap()
    dbg_att = nc.dram_tensor("dbg_att", [128, 2, 512], F32R,
                             kind="ExternalOutput").ap()
    dbg_rstd = nc.dram_tensor("dbg_rstd", [128, 2], F32,
                              kind="ExternalOutput").ap()

    with tile.TileContext(nc) as tc:
        with (
            tc.tile_pool(name="consts", bufs=1) as consts,
            tc.tile_pool(name="dram", bufs=1, space="DRAM") as dram,
        ):
            # ---- constants into SBUF
            c_fw = consts.tile([128, 4, 512], F32R)
            nc.sync.dma_start(c_fw, fw.rearrange("(wc p) n -> p wc n", p=128))
            def ld2(src):
                t = consts.tile([128, 2, H], F32R, tag=f"c_{src.name}")
                nc.sync.dma_start(t, src.rearrange("(hc p) m -> p hc m", p=128))
                return t
            c_dhtr, c_dhti, c_dhtni = ld2(dht_r), ld2(dht_i), ld2(dht_ni)
            c_idr, c_idi, c_idni = ld2(idht_r), ld2(idht_i), ld2(idht_ni)
            c_awr = consts.tile([128, 2, W], F32R, tag="c_awr")
            nc.sync.dma_start(c_awr, awr.rearrange("(jc p) n -> p jc n", p=128))
            c_awi = consts.tile([128, 2, W], F32R, tag="c_awi")
            nc.sync.dma_start(c_awi, awi.rearrange("(jc p) n -> p jc n", p=128))
            c_wgT = consts.tile([128, NCH], mybir.dt.float16, tag="c_wgT")
            nc.sync.dma_start(c_wgT, wgT)
            c_wxT = consts.tile([128, NCH], mybir.dt.float16, tag="c_wxT")
            nc.sync.dma_start(c_wxT, wxT)
            c_bg4 = consts.tile([128, 1], F32, tag="c_bg4")
            nc.sync.dma_start(c_bg4, bg)
            c_bx4 = consts.tile([128, 1], F32, tag="c_bx4")
            nc.sync.dma_start(c_bx4, bx)
            c_gamma = consts.tile([128, NCH], F32, tag="c_gamma")
            nc.sync.dma_start(c_gamma, gamma.to_broadcast([128, NCH]))
            c_beta = consts.tile([128, NCH], F32, tag="c_beta")
            nc.sync.dma_start(c_beta, beta.to_broadcast([128, NCH]))
            c_eps = consts.tile([128, 1], F32, tag="c_eps")
            nc.vector.memset(c_eps, 1e-6)
            c_ident = consts.tile([128, 128], mybir.dt.float16,
                                  tag="c_ident")
            make_identity(nc, c_ident)

            # ---- DRAM scratch
            ysp_g = dram.tile([NCH, H, W], mybir.dt.float16, tag="ysp_g")
            ysp_x = dram.tile([NCH, H, W], mybir.dt.float16, tag="ysp_x")
            r_sp = dram.tile([NCH, H, W], mybir.dt.float16, tag="r_sp")
            cc_in = dram.tile([128, 2 * 1020], F32, tag="cc_in")
            cc_out = dram.tile([128, 2 * 1020], F32, tag="cc_out")

            # ---- Phase A: 1x1 conv, spill y to DRAM
            HB = 32  # h-rows per block
            with (
                tc.tile_pool(name="pa_in", bufs=2) as pa_in,
                tc.tile_pool(name="pa_out", bufs=3) as pa_out,
                tc.tile_pool(name="pa_ps", bufs=4, space="PSUM") as pa_ps,
            ):
                for srct, wTt, biast, yspt in ((gb, c_wgT, c_bg4, ysp_g),
                                               (xb, c_wxT, c_bx4, ysp_x)):
                    for blk in range(H // HB):
                        h0 = blk * HB
                        rh = pa_in.tile([128, HB, W], mybir.dt.float16,
                                        tag="rh")
                        nc.sync.dma_start(rh, srct[:, h0:h0 + HB, :])
                        stag = pa_out.tile([128, HB // 4, W],
                                           mybir.dt.float16, tag="stag")
                        for i2 in range(HB // 4):
                            ps = pa_ps.tile([128, W], F32, tag="cps")
                            for j in range(4):
                                nc.tensor.matmul(
                                    ps[32 * j:32 * (j + 1), :], wTt,
                                    rh[:, i2 * 4 + j, :],
                                    start=True, stop=True,
                                    tile_position=(0, 32 * j))
                            nc.scalar.activation(stag[:, i2, :], ps,
                                                 AF.Identity, bias=biast)
                        for j in range(4):
                            nc.sync.dma_start(
                                yspt[:, h0 + j:h0 + HB:4, :],
                                stag[32 * j:32 * (j + 1), :, :])

            # ---- Phase B: per-channel frequency pipeline
            with (
                tc.tile_pool(name="pb_ld", bufs=3) as pb_ld,
                tc.tile_pool(name="pb_yt", bufs=3) as pb_yt,
                tc.tile_pool(name="pb_yw", bufs=3) as pb_yw,
                tc.tile_pool(name="pb_z", bufs=3) as pb_z,
                tc.tile_pool(name="pb_f", bufs=3) as pb_f,
                tc.tile_pool(name="pb_sm", bufs=3) as pb_sm,
                tc.tile_pool(name="pb_att", bufs=3) as pb_att,
                tc.tile_pool(name="pb_inv", bufs=3) as pb_inv,
                tc.tile_pool(name="pb_r", bufs=2) as pb_r,
                tc.tile_pool(name="pb_acc", bufs=1) as pb_acc,
                tc.tile_pool(name="pb_rl", bufs=3) as pb_rl,
                tc.tile_pool(name="pb_tps", bufs=1, space="PSUM") as pb_tps,
                tc.tile_pool(name="pb_ps", bufs=2, space="PSUM") as pb_ps,
                tc.tile_pool(name="pb_ps5", bufs=3, space="PSUM") as pb_ps5,
            ):
                S1 = pb_acc.tile([128, 2, W], F32, tag="S1")
                S2 = pb_acc.tile([128, 2, W], F32, tag="S2")
                nc.vector.memset(S1, 0.0)
                nc.vector.memset(S2, 0.0)

                for c in range(NCH):
                    z = {}   # (t, 'R'/'I') -> [128, 2(khc), 256]
                    for t, ysp, fPd in ((0, ysp_g, fpg), (1, ysp_x, fpx)):
                        yld = pb_ld.tile([128, 2, W], mybir.dt.float16,
                                         tag="yld")
                        nc.sync.dma_start(
                            yld, ysp[c].rearrange("(hc p) w -> p hc w", p=128))
                        ytT = pb_yt.tile([128, 4, H], F32R, tag="ytT")
                        for hc in range(2):
                            tp = pb_tps.tile([128, 4, 128], mybir.dt.float16,
                                             tag="tp")
                            for wc in range(4):
                                wl = 126 if wc == 3 else 128
                                nc.tensor.transpose(
                                    tp[:wl, wc, :],
                                    yld[:, hc, wc * 128:wc * 128 + wl], c_ident)
                            nc.scalar.copy(
                                out=ytT[:, :, hc * 128:(hc + 1) * 128],
                                in_=tp)
                        # B1: rfft_w -> yw [128(h), hc, kwR|kwI]
                        yw = pb_yw.tile([128, 2, 512], F32R, tag="yw")
                        for hc in range(2):
                            pw = pb_ps5.tile([128, 512], F32, tag="ps512")
                            for wc in range(4):
                                wl = 126 if wc == 3 else 128
                                nc.tensor.matmul(
                                    pw,
                                    _r(ytT[:wl, wc, hc * 128:(hc + 1) * 128]),
                                    _r(c_fw[:wl, wc, :]),
                                    start=(wc == 0), stop=(wc == 3))
                            nc.vector.tensor_copy(out=yw[:, hc, :], in_=pw)
                        # B2: fft_h (complex) + B3 filter
                        fP = pb_f.tile([128, 2, 2, WF], mybir.dt.float16,
                                       tag="fP")
                        nc.sync.dma_start(
                            fP, fPd[c].rearrange("ri (hc p) k -> p ri hc k",
                                                 p=128))
                        fR, fI = fP[:, 0], fP[:, 1]
                        zR = pb_z.tile([128, 2, WF], F32R, tag=f"zR{t}")
                        zI = pb_z.tile([128, 2, WF], F32R, tag=f"zI{t}")
                        for khc in range(2):
                            pfR = pb_ps.tile([128, 256], F32, tag="psA")
                            pfI = pb_ps.tile([128, 256], F32, tag="psB")
                            ksl = slice(khc * 128, (khc + 1) * 128)
                            for hc in range(2):
                                nc.tensor.matmul(
                                    pfR, _r(c_dhtr[:, hc, ksl]),
                                    _r(yw[:, hc, 0:256]),
                                    start=(hc == 0), stop=False)
                                nc.tensor.matmul(
                                    pfR, _r(c_dhtni[:, hc, ksl]),
                                    _r(yw[:, hc, 256:512]),
                                    start=False, stop=(hc == 1))
                                nc.tensor.matmul(
                                    pfI, _r(c_dhti[:, hc, ksl]),
                                    _r(yw[:, hc, 0:256]),
                                    start=(hc == 0), stop=False)
                                nc.tensor.matmul(
                                    pfI, _r(c_dhtr[:, hc, ksl]),
                                    _r(yw[:, hc, 256:512]),
                                    start=False, stop=(hc == 1))
                            t1 = pb_sm.tile([128, WF], F32, tag="t1")
                            t2 = pb_sm.tile([128, WF], F32, tag="t2")
                            t3 = pb_sm.tile([128, WF], F32, tag="t3")
                            t4 = pb_sm.tile([128, WF], F32, tag="t4")
                            nc.vector.tensor_mul(t1, pfR, fR[:, khc, :])
                            nc.vector.tensor_mul(t2, pfI, fI[:, khc, :])
                            nc.gpsimd.tensor_sub(zR[:, khc, :], t1, t2)
                            nc.vector.tensor_mul(t3, pfR, fI[:, khc, :])
                            nc.vector.tensor_mul(t4, pfI, fR[:, khc, :])
                            nc.gpsimd.tensor_add(zI[:, khc, :], t3, t4)
                        z[(t, "R")] = zR
                        z[(t, "I")] = zI

                    gR, gI = z[(0, "R")], z[(0, "I")]
                    xR, xI = z[(1, "R")], z[(1, "I")]
                    # neg imag of x for the scores real part
                    nxI = pb_z.tile([128, 2, WF], F32R, tag="nxI")
                    for khc in range(2):
                        nc.gpsimd.tensor_scalar_mul(
                            nxI[:, khc, :], xI[:, khc, :].bitcast(F32), -1.0)
                    # B4: variance over kw per kh row, rstd = 1/sqrt(2pi var)
                    rstd = pb_sm.tile([128, 2], F32, tag="rstd")
                    for khc in range(2):
                        st = pb_sm.tile([128, 2, 6], F32, tag="bst")
                        nc.vector.bn_stats(out=st[:, 0, :],
                                           in_=gR[:, khc, :].bitcast(F32))
                        nc.vector.bn_stats(out=st[:, 1, :],
                                           in_=gI[:, khc, :].bitcast(F32))
                        mvR = pb_sm.tile([128, 2], F32, tag="mvR")
                        mvI = pb_sm.tile([128, 2], F32, tag="mvI")
                        nc.vector.bn_aggr(out=mvR, in_=st[:, 0, :])
                        nc.vector.bn_aggr(out=mvI, in_=st[:, 1, :])
                        vs = pb_sm.tile([128, 1], F32, tag="vs")
                        nc.vector.tensor_add(vs, mvR[:, 1:2], mvI[:, 1:2])
                        # vs2 = 2*pi*var, explicitly (ACT scale is unreliable)
                        vs2 = pb_sm.tile([128, 1], F32, tag="vs2")
                        nc.vector.tensor_scalar_mul(vs2, vs,
                                                    float(2.0 * np.pi))
                        sd = pb_sm.tile([128, 1], F32, tag="sd")
                        nc.scalar.activation(sd, vs2, AF.Sqrt)
                        y0 = pb_sm.tile([128, 1], F32, tag="y0")
                        nc.vector.reciprocal(y0, sd)
                        # one Newton step: y1 = y0*(1.5 - 0.5*vs2*y0^2)
                        t_n = pb_sm.tile([128, 1], F32, tag="t_n")
                        nc.vector.tensor_mul(t_n, y0, y0)
                        nc.vector.tensor_mul(t_n, t_n, vs2)
                        nc.vector.tensor_scalar(
                            out=t_n, in0=t_n,
                            scalar1=-0.5, scalar2=1.5,
                            op0=mybir.AluOpType.mult,
                            op1=mybir.AluOpType.add)
                        nc.vector.tensor_mul(rstd[:, khc:khc + 1], y0, t_n)
                    if c == 0:
                        nc.sync.dma_start(dbg_zg[:, :, 0:256], gR)
                        nc.sync.dma_start(dbg_zg[:, :, 256:512], gI)
                        nc.sync.dma_start(dbg_rstd, rstd)
                    # B5 scores + B6 sigmoid with fused row scale
                    att = pb_att.tile([128, 2, 512], F32R, tag="att")
                    for ic in range(2):
                        pscR = pb_ps.tile([128, 256], F32, tag="psA")
                        pscI = pb_ps.tile([128, 256], F32, tag="psB")
                        isl = slice(ic * 128, (ic + 1) * 128)
                        for khc in range(2):
                            nc.tensor.matmul(pscR,
                                             _r(gR[:, khc, isl]),
                                             _r(xR[:, khc, :]),
                                             start=(khc == 0), stop=False)
                            nc.tensor.matmul(pscR,
                                             _r(gI[:, khc, isl]),
                                             _r(nxI[:, khc, :]),
                                             start=False, stop=(khc == 1))
                            nc.tensor.matmul(pscI,
                                             _r(gR[:, khc, isl]),
                                             _r(xI[:, khc, :]),
                                             start=(khc == 0), stop=False)
                            nc.tensor.matmul(pscI,
                                             _r(gI[:, khc, isl]),
                                             _r(xR[:, khc, :]),
                                             start=False, stop=(khc == 1))
                        nc.scalar.activation(att[:, ic, 0:256], pscR,
                                             AF.Sigmoid,
                                             scale=rstd[:, ic:ic + 1])
                        nc.scalar.activation(att[:, ic, 256:512], pscI,
                                             AF.Sigmoid,
                                             scale=rstd[:, ic:ic + 1])
                    if c == 0:
                        nc.sync.dma_start(dbg_att, att)
                    # B7: ifft over i, output transposed [j, mR|mI]
                    inv = pb_inv.tile([128, 2, 512], F32R, tag="inv")
                    for jc in range(2):
                        pvR = pb_ps.tile([128, 256], F32, tag="psA")
                        pvI = pb_ps.tile([128, 256], F32, tag="psB")
                        jsl = slice(jc * 128, (jc + 1) * 128)
                        jsl2 = slice(256 + jc * 128, 256 + (jc + 1) * 128)
                        for ic in range(2):
                            nc.tensor.matmul(pvR,
                                             _r(att[:, ic, jsl]),
                                             _r(c_idr[:, ic, :]),
                                             start=(ic == 0), stop=False)
                            nc.tensor.matmul(pvR,
                                             _r(att[:, ic, jsl2]),
                                             _r(c_idni[:, ic, :]),
                                             start=False, stop=(ic == 1))
                            nc.tensor.matmul(pvI,
                                             _r(att[:, ic, jsl]),
                                             _r(c_idi[:, ic, :]),
                                             start=(ic == 0), stop=False)
                            nc.tensor.matmul(pvI,
                                             _r(att[:, ic, jsl2]),
                                             _r(c_idr[:, ic, :]),
                                             start=False, stop=(ic == 1))
                        nc.scalar.copy(out=inv[:, jc, 0:256], in_=pvR)
                        nc.scalar.copy(out=inv[:, jc, 256:512], in_=pvI)
                    # B8 irfft over j + B9 residual & stats
                    rc2 = pb_r.tile([128, 2, W], mybir.dt.float16,
                                    tag="rc2")
                    xc2 = pb_r.tile([128, 2, W], F32, tag="xc2")
                    nc.sync.dma_start(
                        xc2, xres[c].rearrange("(hc p) w -> p hc w", p=128))
                    for mc in range(2):
                        pr = pb_ps5.tile([128, W], F32, tag="ps512")
                        msl = slice(mc * 128, (mc + 1) * 128)
                        msl2 = slice(256 + mc * 128, 256 + (mc + 1) * 128)
                        for jc in range(2):
                            nc.tensor.matmul(pr, _r(inv[:, jc, msl]),
                                             _r(c_awr[:, jc, :]),
                                             start=(jc == 0), stop=False)
                            nc.tensor.matmul(pr, _r(inv[:, jc, msl2]),
                                             _r(c_awi[:, jc, :]),
                                             start=False, stop=(jc == 1))
                        rc = rc2[:, mc, :]
                        nc.vector.tensor_add(rc, pr, xc2[:, mc, :])
                        nc.vector.tensor_add(S1[:, mc, :], S1[:, mc, :], rc)
                        sq = pb_r.tile([128, W], F32, tag="sq")
                        nc.gpsimd.tensor_mul(sq, rc, rc)
                        nc.gpsimd.tensor_add(S2[:, mc, :], S2[:, mc, :], sq)
                    nc.sync.dma_start(
                        r_sp[c].rearrange("(hc p) w -> p hc w", p=128), rc2)

                # ---- Phase C: LN stats AllReduce + normalize
                nc.sync.dma_start(cc_in[:, 0:1020],
                                  S1.rearrange("p a b -> p (a b)"))
                nc.sync.dma_start(cc_in[:, 1020:2040],
                                  S2.rearrange("p a b -> p (a b)"))
                nc.gpsimd.collective_compute(
                    "AllReduce", mybir.AluOpType.add,
                    replica_groups=[[0, 1, 2, 3], [4, 5, 6, 7]],
                    ins=[cc_in.opt()], outs=[cc_out.opt()])
                mu = pb_acc.tile([128, 1020], F32, tag="mu")
                var = pb_acc.tile([128, 1020], F32, tag="var")
                nc.sync.dma_start(mu, cc_out[:, 0:1020])
                nc.sync.dma_start(var, cc_out[:, 1020:2040])
                nc.scalar.mul(out=mu, in_=mu, mul=1.0 / C)
                nc.scalar.mul(out=var, in_=var, mul=1.0 / C)
                ta = pb_acc.tile([128, 1020], F32, tag="ta")
                nc.vector.tensor_mul(ta, mu, mu)
                nc.vector.tensor_sub(var, var, ta)  # var = E[r^2]-mu^2
                nc.vector.tensor_scalar_add(var, var, 1e-6)
                nc.scalar.activation(ta, var, AF.Sqrt)
                y0l = pb_acc.tile([128, 1020], F32, tag="y0l")
                nc.vector.reciprocal(y0l, ta)
                nc.vector.tensor_mul(ta, y0l, y0l)
                nc.vector.tensor_mul(ta, ta, var)
                nc.vector.tensor_scalar(
                    out=ta, in0=ta, scalar1=-0.5, scalar2=1.5,
                    op0=mybir.AluOpType.mult, op1=mybir.AluOpType.add)
                rstd_ln = pb_acc.tile([128, 1020], F32, tag="rstd_ln")
                nc.vector.tensor_mul(rstd_ln, y0l, ta)
                for c0 in range(0, NCH, 4):
                    for mc in range(2):
                        msl = slice(mc * 128, (mc + 1) * 128)
                        wsl = slice(mc * W, (mc + 1) * W)
                        rl4 = pb_rl.tile([128, 4, W], mybir.dt.float16,
                                         tag="rl4")
                        nc.sync.dma_start(
                            rl4, r_sp[c0:c0 + 4, msl, :].rearrange(
                                "c p w -> p c w"))
                        ob4 = pb_r.tile([128, 4, W], F32, tag="ob4")
                        for ci in range(4):
                            tt = pb_r.tile([128, W], F32, tag="tt")
                            nc.gpsimd.tensor_sub(tt, rl4[:, ci, :], mu[:, wsl])
                            nc.vector.tensor_mul(tt, tt, rstd_ln[:, wsl])
                            nc.vector.tensor_scalar(
                                out=ob4[:, ci, :], in0=tt,
                                scalar1=c_gamma[:, c0 + ci:c0 + ci + 1],
                                scalar2=c_beta[:, c0 + ci:c0 + ci + 1],
                                op0=mybir.AluOpType.mult,
                                op1=mybir.AluOpType.add)
                        nc.sync.dma_start(
                            out[c0:c0 + 4, msl, :].rearrange("c p w -> p c w"),
                            ob4)
    nc.compile()
    return nc


_PROGRAM = None


def kernel(_trace=False, **inputs):
    global _PROGRAM
    np_in = {k: np.ascontiguousarray(np.asarray(v)) for k, v in inputs.items()}
    g, x = np_in["g"], np_in["x"]
    consts = build_consts()
    in_maps = []
    for k in range(N_CORES):
        b, grp = k // 4, k % 4
        sl = slice(grp * NCH, (grp + 1) * NCH)
        m = dict(
            gb=np.ascontiguousarray(g[b].astype(np.float16)),
            xb=np.ascontiguousarray(x[b].astype(np.float16)),
            xres=np.ascontiguousarray(x[b][sl]),
            wgT=np.ascontiguousarray(np_in["wg_conv"][sl].T.astype(np.float16)),
            wxT=np.ascontiguousarray(np_in["wx_conv"][sl].T.astype(np.float16)),
            bg=np.ascontiguousarray(np.tile(np_in["bg_conv"][sl], 4)[:, None]),
            bx=np.ascontiguousarray(np.tile(np_in["bx_conv"][sl], 4)[:, None]),
            fpg=np.ascontiguousarray(np.moveaxis(
                np_in["filt_g"][sl], 3, 1).astype(np.float16)),
            fpx=np.ascontiguousarray(np.moveaxis(
                np_in["filt_x"][sl], 3, 1).astype(np.float16)),
            gamma=np.ascontiguousarray(np_in["ln_gamma"][sl][None, :]),
            beta=np.ascontiguousarray(np_in["ln_beta"][sl][None, :]),
            **consts,
        )
        in_maps.append(m)
    if _PROGRAM is None:
        _PROGRAM = build_program()
    res = run_bass_kernel_spmd(_PROGRAM, in_maps, core_ids=list(range(N_CORES)),
                               trace=_trace)
    out = np.zeros((B, C, H, W), np.float32)
    for k in range(N_CORES):
        b, grp = k // 4, k % 4
        out[b, grp * NCH:(grp + 1) * NCH] = res.results[k]["out"]
    kernel.last_debug = {k2: res.results[0][k2]
                         for k2 in ("dbg_zg", "dbg_att", "dbg_rstd")
                         if k2 in res.results[0]}
    if _trace:
        kernel.last_results = res
    return out


if __name__ == "__main__":
    ins = {
        "g": np.random.randn(B, C, H, W).astype(np.float32),
        "x": np.random.randn(B, C, H, W).astype(np.float32),
        "wg_conv": (np.random.randn(C, C) * 0.05).astype(np.float32),
        "bg_conv": np.zeros(C, np.float32),
        "wx_conv": (np.random.randn(C, C) * 0.05).astype(np.float32),
        "bx_conv": np.zeros(C, np.float32),
        "filt_g": (np.random.randn(C, H, WF, 2) * 0.02).astype(np.float32),
        "filt_x": (np.random.randn(C, H, WF, 2) * 0.02).astype(np.float32),
        "ln_gamma": np.ones(C, np.float32),
        "ln_beta": np.zeros(C, np.float32),
    }
    o = kernel(**ins)
    print("kernel ran, out shape", o.shape)

